# revision 5
# baseline (speedup 1.0000x reference)
"""Single-head attention (B=4, S=4096, D=512, H=64) on 8 TRN2 NeuronCores.

Sharding: core c handles batch b=c//2 and key-half h=c%2 (2048 keys), for ALL
4096 queries of that batch. Softmax uses a constant shift (exp(s/8 - 2)) so
per-key-half partial numerators/denominators are directly addable; the host
merges the partials per batch and normalizes.

Engine split (the exp of 2048x4096 scores is the wall; PE work is hidden
under it via row-group-paired score matmuls and 2-chain attn accumulation):
  ACT: exact Exp on half the score tiles + head-phase projection copies
  DVE: bitcast-linear exp (i16 = round(A*s+B) reinterpreted as fp16 ==
       2^(A*s+B-15360)/1024-ish, the Schraudolph trick) on the other half,
       via a fast 2-stage form (psum->f16 mult/add, then f16->i16 convert,
       which is exact since fp16 values >= 2048 are integers), plus the
       per-m-tile PSUM->SBUF staging of the two attn accumulator chains.
  PE : scores k^T x q^T in 64-row-group pairs; attn [v|1]^T x P^T in two
       interleaved accumulation chains (chains stall ~330ns on back-to-back
       same-accumulator matmuls; alternating chains hides it).
Key-projection bias cancels in softmax (constant per query) and the v bias
commutes through the weighted average (out = num/den + bv), so neither is
computed on device.

Device layout (per core):
  xt  [512, 4096] fp16  -- x[b]^T with columns rotated so the core's key half
                           is columns 0..2047 (queries therefore permuted too)
  out [130, 4096] fp16  -- rows 0..64: chain-A partial [num^T; den], rows
                           65..129: chain-B partial; host sums A+B+other core.
"""

import numpy as np

import concourse.bass as bass
import concourse.tile as tile
import concourse.mybir as mybir
from concourse import bass_utils

B, S, D, H = 4, 4096, 512, 64
N_CORES = 8
KC = S // 2          # keys per core
NCHUNK = KC // 128   # 16 key chunks of 128
NPAIR = NCHUNK // 2  # 8 chunk pairs
MT = 512             # query tile width
NM = S // MT         # 8 query tiles
VW = H + 1           # v columns + ones column
EXP_SHIFT = -2.0     # constant softmax shift; cancels in normalization

# Schraudolph constants: i16 = round(SCH_A*s + SCH_B); bitcast fp16 ~ exp(s/8-2)
LOG2E = 1.4426950408889634
SCH_A = 1024.0 * LOG2E * 0.125
SCH_B = 1024.0 * (15.0 + EXP_SHIFT * LOG2E) - 0.0573 * 1024.0

# Per-pair exp method for each of the 8 pairs in an m-tile ('a'=ACT, 'd'=DVE).
METHODS = ['d', 'a', 'd', 'a', 'd', 'a', 'd', 'a']

f16 = mybir.dt.float16
f32 = mybir.dt.float32
i16 = mybir.dt.int16
Alu = mybir.AluOpType
AF = mybir.ActivationFunctionType


_SELF_CLOCK = {
    "EngineType.Activation": "Activation_",
    "EngineType.DVE": "DVE_",
    "EngineType.PE": "PE_",
}
_DROP_SELF_WAITS = [True]  # list so profiling scripts can toggle


def _split_multi_waits(nc):
    """This walrus build encodes at most ONE sync-wait command per
    instruction. First drop same-engine clock waits (trivially satisfied on
    an in-order serially-executing engine: the waited value corresponds to an
    instruction that must have completed before this one can start), then
    hoist surplus waits onto standalone single-wait EventSemaphore
    instructions inserted just before the original."""
    ctr = 0
    for f in nc.m.functions:
        for bb in f.blocks:
            insts = bb.instructions
            i = 0
            while i < len(insts):
                inst = insts[i]
                si = inst.sync_info
                if si is not None and si.on_wait and len(si.on_wait) > 1:
                    pfx = _SELF_CLOCK.get(str(inst.engine)) if _DROP_SELF_WAITS[0] else None
                    waits = list(si.on_wait)
                    if pfx is not None:
                        kept = [
                            w for w in waits
                            if not (w.ant_name or "").startswith(pfx)
                        ]
                        if kept:  # never drop ALL waits
                            waits = kept
                    if len(waits) == 1:
                        inst.sync_info = mybir.SyncInfo(
                            on_wait=waits, on_update=list(si.on_update or [])
                        )
                        i += 1
                        continue
                    inst.sync_info = mybir.SyncInfo(
                        on_wait=[waits[-1]], on_update=list(si.on_update or [])
                    )
                    for w in waits[:-1]:
                        ev = mybir.InstEventSemaphore(
                            name=f"W-split-{ctr}", ins=[], outs=[]
                        )
                        ctr += 1
                        ev.engine = inst.engine
                        ev.sync_info = mybir.SyncInfo(on_wait=[w], on_update=[])
                        insts.insert(i, ev)
                        i += 1
                i += 1
    return ctr


def _trim_tail_barrier(nc):
    """Drop the second all-engine barrier after the tail sem-clear: each
    engine's stream already synchronized at barrier 1, the Pool-side InstISA
    clear runs after it, and nothing but stream-end follows. Saves ~2 us of
    end-of-kernel EVSEM butterfly."""
    for f in nc.m.functions:
        for bb in f.blocks:
            if not bb.name.endswith("_end"):
                continue
            insts = bb.instructions
            last_isa = None
            for i, inst in enumerate(insts):
                if type(inst).__name__ == "InstISA":
                    last_isa = i
            if last_isa is not None:
                while len(insts) > last_isa + 1:
                    insts.pop()


def _build_nc(reps=1, phase="full"):
    nc = bass.Bass("TRN2", target_bir_lowering=False, debug=False)
    xt = nc.dram_tensor("xt", [D, S], f16, kind="ExternalInput").ap()
    wq = nc.dram_tensor("wq", [D, 128], f16, kind="ExternalInput").ap()
    wk = nc.dram_tensor("wk", [D, 128], f16, kind="ExternalInput").ap()
    wv = nc.dram_tensor("wv", [D, VW], f16, kind="ExternalInput").ap()
    bq = nc.dram_tensor("bq", [128, 1], f32, kind="ExternalInput").ap()
    out = nc.dram_tensor("out", [2 * VW, S], f16, kind="ExternalOutput").ap()

    NDC = D // 128  # 4 contraction chunks

    def body(tc, rep, xsb, qkv, ptp, oip, outsb, wq_sb, wk_sb, wv_sb,
             bq_sb, shift_sb):
        # --- x^T load in query-column blocks (all 4 d-chunks per block) so
        # projections pipeline under the DMA ---
        x_sb = xsb.tile([128, NDC, S], f16, tag="x")
        xt_r = xt.rearrange("(c p) s -> p c s", p=128)
        for blk in range(NM):
            bs = blk * MT
            if blk == 0:
                # per-chunk DMAs so the first projection matmul starts as soon
                # as d-chunk 0 lands (shortens the kernel-head critical chain)
                for c in range(NDC):
                    nc.sync.dma_start(
                        out=x_sb[:, c, bs:bs + MT], in_=xt_r[:, c, bs:bs + MT]
                    )
            else:
                nc.sync.dma_start(
                    out=x_sb[:, :, bs:bs + MT], in_=xt_r[:, :, bs:bs + MT]
                )
        if phase == "dma":
            return

        # --- projections: 2-chain interleaved accumulation, ACT copies ---
        qTd_sb = qkv.tile([128, S], f16, tag="qT")  # q^T dup'd on parts 64..127
        kTd_sb = qkv.tile([128, KC], f16, tag="kT")  # k^T (core's keys) dup'd
        v_sb = qkv.tile([128, NCHUNK * VW], f16, tag="v")  # [v|1] chunk-packed

        with (
            tc.tile_pool(name=f"pproj{rep}", bufs=1, space="PSUM") as pproj,
            tc.tile_pool(name=f"pvp{rep}", bufs=2, space="PSUM") as pvp,
        ):
            # Block-quad projections: one [128, 4*MT] psum tile spans 4 banks
            # (one full bank per 512-col block, so the 4 accumulation chains
            # never share a bank) and each d-chunk weight load feeds 4 moving
            # streams instead of 2, quartering the Ldweights reload count.
            QB = 4 * MT
            pk = pproj.tile([128, QB], f32, tag="pq", name="pk")
            for c in range(NDC):
                for blk in range(4):  # key blocks 0..3
                    nc.tensor.matmul(
                        pk[:, blk * MT:(blk + 1) * MT],
                        lhsT=wk_sb[:, c, :],
                        rhs=x_sb[:, c, blk * MT:(blk + 1) * MT],
                        start=(c == 0), stop=(c == NDC - 1),
                    )
            # k^T staging on ACT (it is idle here until the q blocks land);
            # DVE keeps the v copies so the two head-phase copy streams run
            # in parallel
            nc.scalar.activation(out=kTd_sb, in_=pk, func=AF.Identity)
            for sp in range(8):  # v chunk pairs
                s0, s1 = 2 * sp, 2 * sp + 1
                # separate tiles: PSUM accumulation start resets at bank
                # granularity, so chains must not share a bank
                pv0 = pvp.tile([128, VW], f32, tag="pv", name="pv0")
                pv1 = pvp.tile([128, VW], f32, tag="pv", name="pv1")
                for c in range(NDC):
                    for pv, s_ in ((pv0, s0), (pv1, s1)):
                        nc.tensor.matmul(
                            pv,
                            lhsT=x_sb[:, c, s_ * 128:(s_ + 1) * 128],
                            rhs=wv_sb[:, c, :],
                            start=(c == 0), stop=(c == NDC - 1),
                        )
                for pv, s_ in ((pv0, s0), (pv1, s1)):
                    nc.vector.tensor_copy(
                        out=v_sb[:, s_ * VW:s_ * VW + H],
                        in_=pv[:, 0:H],
                    )
            for qq in range(2):  # query block quads (0-3, 4-7)
                pq = pproj.tile([128, QB], f32, tag="pq")
                for c in range(NDC):
                    for blk in range(4 * qq, 4 * qq + 4):
                        nc.tensor.matmul(
                            pq[:, (blk % 4) * MT:(blk % 4 + 1) * MT],
                            lhsT=wq_sb[:, c, :],
                            rhs=x_sb[:, c, blk * MT:(blk + 1) * MT],
                            start=(c == 0), stop=(c == NDC - 1),
                        )
                nc.scalar.activation(
                    out=qTd_sb[:, 4 * qq * MT:(4 * qq + 4) * MT], in_=pq,
                    func=AF.Identity, bias=bq_sb,
                )
        # ones column (denominator): strided memset over the 16 chunk slots
        ones_view = v_sb[:, :].rearrange("p (c w) -> p c w", w=VW)[:, :, H:H + 1]
        nc.vector.memset(ones_view, 1.0)
        if phase == "proj":
            return

        # --- main attention loop ---
        with (
            tc.tile_pool(name=f"psc{rep}", bufs=3, space="PSUM") as pscp,
            tc.tile_pool(name=f"pout{rep}", bufs=1, space="PSUM") as poutp,
        ):
            # Flat software pipeline over all (m, j) slots: the pending attn
            # queue carries across m-tile boundaries so the PE/exp stream
            # never drains (the per-m flush cost ~1.5-2.5us of engine bubble
            # at every boundary).
            LAG = 3
            po_tiles = {}   # m -> (po_a, po_b)
            pending = []    # (m, pt, c0)

            def drain_one():
                m_, pt_, c0_ = pending.pop(0)
                po_a, po_b = po_tiles[m_]
                _emit_attn(nc, po_a, po_b, v_sb, pt_, c0_)
                if c0_ == NCHUNK - 2:  # last pair of m_: stage + DMA out
                    # staging split across engines: chain A on DVE, chain B on
                    # ACT, so neither exp engine eats both 658ns copies
                    oa_sb = outsb.tile([VW, MT], f16, tag="oa")
                    ob_sb = outsb.tile([VW, MT], f16, tag="ob")
                    nc.vector.tensor_copy(out=oa_sb, in_=po_a)
                    nc.scalar.activation(out=ob_sb, in_=po_b, func=AF.Identity)
                    ms_ = m_ * MT
                    nc.sync.dma_start(out=out[0:VW, ms_:ms_ + MT], in_=oa_sb)
                    nc.sync.dma_start(out=out[VW:2 * VW, ms_:ms_ + MT], in_=ob_sb)
                    del po_tiles[m_]

            for m in range(NM):
                ms = m * MT
                for j in range(NPAIR):
                    c0 = 2 * j
                    psc = pscp.tile([128, 2 * MT], f32, tag="psc")
                    for q in range(2):
                        ck = c0 + q
                        rb = 64 * (ck % 2)  # alternate PE row groups
                        nc.tensor.matmul(
                            psc[:, q * MT:(q + 1) * MT],
                            lhsT=kTd_sb[rb:rb + 64, ck * 128:(ck + 1) * 128],
                            rhs=qTd_sb[rb:rb + 64, ms:ms + MT],
                            start=True, stop=True,
                        )
                    if phase == "scores":
                        continue
                    if METHODS[j] == 'a':
                        pt = ptp.tile([128, 2 * MT], f16, tag="pt")
                        nc.scalar.activation(
                            out=pt, in_=psc, func=AF.Exp,
                            bias=shift_sb, scale=0.125,
                        )
                    else:
                        # single-op Schraudolph: f32 PSUM -> i16 SBUF with the
                        # affine fused into the convert (round-to-nearest on
                        # the f32->i16 write; more accurate than the old
                        # 2-stage f16 staging, which quantized A*s+B onto the
                        # f16 grid: +-8 i16 ulp above 16384)
                        oi = oip.tile([128, 2 * MT], i16, tag="oi")
                        nc.vector.tensor_scalar(
                            out=oi, in0=psc, scalar1=SCH_A, scalar2=SCH_B,
                            op0=Alu.mult, op1=Alu.add,
                        )
                        pt = oi[:, :].bitcast(f16)
                    if phase == "exp":
                        continue
                    if j == 0:
                        po_tiles[m] = (
                            poutp.tile([VW, MT], f32, tag="poa", name="po_a"),
                            poutp.tile([VW, MT], f32, tag="pob", name="po_b"),
                        )
                    pending.append((m, pt, c0))
                    if len(pending) > LAG:
                        drain_one()
            if phase not in ("scores", "exp"):
                while pending:
                    drain_one()

    with tile.TileContext(nc) as tc:
        with (
            tc.tile_pool(name="consts", bufs=1) as consts,
            tc.tile_pool(name="xsb", bufs=1) as xsb,
            tc.tile_pool(name="qkv", bufs=2) as qkv,
            tc.tile_pool(name="pt", bufs=4) as ptp,
            tc.tile_pool(name="oo", bufs=3) as oip,
            tc.tile_pool(name="outsb", bufs=2) as outsb,
        ):
            # --- constants ---
            wq_sb = consts.tile([128, NDC, 128], f16)
            wk_sb = consts.tile([128, NDC, 128], f16)
            wv_sb = consts.tile([128, NDC, VW], f16)
            nc.sync.dma_start(out=wq_sb, in_=wq.rearrange("(c p) m -> p c m", p=128))
            nc.sync.dma_start(out=wk_sb, in_=wk.rearrange("(c p) m -> p c m", p=128))
            nc.sync.dma_start(out=wv_sb, in_=wv.rearrange("(c p) m -> p c m", p=128))
            bq_sb = consts.tile([128, 1], f32)
            nc.sync.dma_start(out=bq_sb, in_=bq)
            shift_sb = consts.tile([128, 1], f32)
            nc.vector.memset(shift_sb, EXP_SHIFT)

            for rep in range(reps):
                body(tc, rep, xsb, qkv, ptp, oip, outsb, wq_sb, wk_sb,
                     wv_sb, bq_sb, shift_sb)

    _split_multi_waits(nc)
    _trim_tail_barrier(nc)
    return nc


ATTN_MODE = "il2"  # "il2": full-K two chains | "rowsplit": K=64 row groups
BITCAST_SIDE = "rhs"  # "out": bitcast on DVE copy output | "rhs": on matmul rhs


def _emit_attn(nc, po_a, po_b, v_sb, pt, c0):
    if ATTN_MODE == "il2":
        for q in range(2):
            ck = c0 + q
            po = po_a if q == 0 else po_b
            nc.tensor.matmul(
                po, lhsT=v_sb[:, ck * VW:(ck + 1) * VW],
                rhs=pt[:, q * MT:(q + 1) * MT],
                start=(ck < 2), stop=(ck >= NCHUNK - 2),
            )
    else:
        for q in range(2):
            ck = c0 + q
            for half, po in ((0, po_a), (1, po_b)):
                rb = 64 * half
                nc.tensor.matmul(
                    po, lhsT=v_sb[rb:rb + 64, ck * VW:(ck + 1) * VW],
                    rhs=pt[rb:rb + 64, q * MT:(q + 1) * MT],
                    start=(ck == 0), stop=(ck == NCHUNK - 1),
                )


_NC_CACHE = []


def _prepare_in_maps(x, Wq, bq, Wk, bk, Wv, bv):
    x = np.asarray(x, dtype=np.float32)
    Wq, Wk, Wv = (np.asarray(a, dtype=np.float32) for a in (Wq, Wk, Wv))
    bq = np.asarray(bq, dtype=np.float32)

    wq_dup = np.concatenate([Wq, Wq], axis=1).astype(np.float16)      # [512,128]
    wk_dup = np.concatenate([Wk, Wk], axis=1).astype(np.float16)
    wv_aug = np.concatenate([Wv, np.zeros((D, 1), np.float32)], axis=1).astype(np.float16)
    bq_dup = np.concatenate([bq, bq]).astype(np.float32).reshape(128, 1)

    in_maps = []
    for c in range(N_CORES):
        b, h = c // 2, c % 2
        xt_b = np.ascontiguousarray(x[b].T)  # [512, 4096]
        if h == 1:
            xt_b = np.roll(xt_b, -KC, axis=1)  # core's key half first
        in_maps.append({
            "xt": xt_b.astype(np.float16),
            "wq": wq_dup, "wk": wk_dup, "wv": wv_aug,
            "bq": bq_dup,
        })
    return in_maps


def _merge_results(results, bv):
    bv = np.asarray(bv, dtype=np.float64)
    out = np.empty((B, S, H), dtype=np.float32)
    for b in range(B):
        a = results[2 * b]["out"].astype(np.float64)       # natural q order
        bb = results[2 * b + 1]["out"].astype(np.float64)  # q order rolled by -KC
        bb = np.roll(bb, KC, axis=1)                       # undo the roll
        tot = a[:VW] + a[VW:] + bb[:VW] + bb[VW:]
        out[b] = ((tot[:H, :] / tot[H:H + 1, :]).T + bv).astype(np.float32)
    return out


def kernel(x, Wq, bq, Wk, bk, Wv, bv):
    in_maps = _prepare_in_maps(x, Wq, bq, Wk, bk, Wv, bv)
    if not _NC_CACHE:
        _NC_CACHE.append(_build_nc())
    nc = _NC_CACHE[0]
    res = bass_utils.run_bass_kernel_spmd(nc, in_maps, core_ids=list(range(N_CORES)))
    return _merge_results(res.results, bv)

